# revision 2
# baseline (speedup 1.0000x reference)
"""ChebConv (K=3) Trainium2 kernel v8.

vs v7 (trace: transposes pipeline at ~56ns/tile when fed; the stream is
paced by PSUM evacuations and the early DMA chain):
  - PSUM transpose groups are 8 tiles / 2 banks -> ONE evac per group,
    2 per strip (halves DVE/ACT evac op count; drain amortizes).
  - All transient PSUM (transpose groups, y0d groups, Q chunks) lives in
    one 2-buffer pool; z1/z2 accumulators keep the other 4 banks.
  - x loads per strip (64KB each) instead of one 1MB strided DMA that
    stalled the ring; adj strips alternate between both HWDGE queues.
  - Dense tail: all 64 pass-2 matmuls back-to-back (evacs slot in at
    chunk boundaries), then Q chunks, then 2x(8 transposes + combine +
    512KB store).
"""

import numpy as np

B, N, F, K = 8, 2048, 128, 3
P = 128
NT = N // P  # 16
NCORES = 8
BUNDLES = [(0, 4), (4, 4), (8, 4), (12, 3), (15, 1)]
NB2 = 4

_cache = {}


def _build_nc():
    from contextlib import ExitStack

    import concourse.bacc as bacc
    import concourse.tile as tile
    from concourse import mybir

    f32 = mybir.dt.float32
    f16 = mybir.dt.float16
    AF = mybir.ActivationFunctionType
    OP = mybir.AluOpType

    nc = bacc.Bacc("TRN2", target_bir_lowering=False, debug=False, num_devices=NCORES)
    adj = nc.dram_tensor("adj", [N, N], f32, kind="ExternalInput").ap()
    x = nc.dram_tensor("x", [N, F], f32, kind="ExternalInput").ap()
    Wt = nc.dram_tensor("Wt", [K, F, F], f32, kind="ExternalInput").ap()
    bsum_d = nc.dram_tensor("bsum", [P, F], f32, kind="ExternalInput").ap()
    ident = nc.dram_tensor("ident", [P, P], f32, kind="ExternalInput").ap()
    out = nc.dram_tensor("out", [N, F], f32, kind="ExternalOutput").ap()
    out_t = out.rearrange("(t p) f -> p t f", p=P)

    with ExitStack() as ctx:
        tc = ctx.enter_context(tile.TileContext(nc))
        consts = ctx.enter_context(tc.tile_pool(name="consts", bufs=1))
        afp = ctx.enter_context(tc.tile_pool(name="afp", bufs=5))
        abp = ctx.enter_context(tc.tile_pool(name="abp", bufs=4))
        big = ctx.enter_context(tc.tile_pool(name="big", bufs=1))
        small = ctx.enter_context(tc.tile_pool(name="small", bufs=4))
        ps_acc = ctx.enter_context(tc.tile_pool(name="ps_acc", bufs=1, space="PSUM"))
        ps_t = ctx.enter_context(tc.tile_pool(name="ps_t", bufs=2, space="PSUM"))

        def strip_q(r):
            return nc.sync if r % 2 == 0 else nc.scalar

        # ---- first DMAs ------------------------------------------------
        afs = [afp.tile([P, N], f32, tag="af", name=f"af{r}") for r in range(2)]
        strip_q(0).dma_start(out=afs[0], in_=adj[0:P, :])
        strip_q(1).dma_start(out=afs[1], in_=adj[P:2 * P, :])
        ident_hf = consts.tile([P, P], f16)
        nc.gpsimd.dma_start(out=ident_hf, in_=ident)
        bsum = consts.tile([P, F], f32)
        nc.sync.dma_start(out=bsum, in_=bsum_d)
        w_hf = consts.tile([P, K, F], f16)
        nc.gpsimd.dma_start(out=w_hf, in_=Wt.rearrange("k i o -> i k o"))
        xts = [small.tile([P, F], f32, tag="xt", name=f"xt{r}", bufs=6)
               for r in range(2)]
        nc.sync.dma_start(out=xts[0], in_=x[0:P, :])
        nc.scalar.dma_start(out=xts[1], in_=x[P:2 * P, :])

        # ---- HAM warm-up ----------------------------------------------
        warm_ps = ps_t.tile([P, 8, P], f32, tag="t")
        for _ in range(24):
            nc.tensor.matmul(warm_ps[:, 0, :], lhsT=ident_hf, rhs=ident_hf,
                             start=True, stop=True)

        dsq = consts.tile([P, NT], f32)
        dinv = consts.tile([P, NT], f32)
        ndinv2 = consts.tile([P, NT], f32)

        y0 = big.tile([P, NT, F], f16)
        y0d = big.tile([P, NT, F], f16)
        y1 = big.tile([P, NT, F], f16)
        ats = big.tile([P, NT, N], f16)
        y0dT = big.tile([P, N], f16)
        z1bf = big.tile([P, N], f16)
        z2bf = big.tile([P, N], f16)
        qbf = big.tile([P, N], f16)

        z1t = ps_acc.tile([P, N], f32, tag="acc")

        def evac(use_act, out, in_):
            if use_act:
                nc.scalar.copy(out=out, in_=in_)
            else:
                nc.vector.tensor_copy(out=out, in_=in_)

        bnd_of_strip = {}
        for bi, (lo, n) in enumerate(BUNDLES):
            for r in range(lo, lo + n):
                bnd_of_strip[r] = bi

        ats_list = [None] * NT
        d_los = [None] * NT
        d_his = [None] * NT

        def do_cast(r):
            a_t = abp.tile([P, N], f16, tag="a", name=f"at{r}")
            d_lo = small.tile([P, 1], f32, tag="dlo", name=f"dlo{r}")
            d_hi = small.tile([P, 1], f32, tag="dhi", name=f"dhi{r}")
            nc.vector.tensor_scalar(
                out=a_t[:, 0:N // 2], in0=afs[r][:, 0:N // 2], scalar1=1.0,
                scalar2=0.0, op0=OP.mult, op1=OP.add, accum_out=d_lo)
            nc.scalar.activation(out=a_t[:, N // 2:N], in_=afs[r][:, N // 2:N],
                                 func=AF.Identity, accum_out=d_hi)
            ats_list[r] = a_t
            d_los[r], d_his[r] = d_lo, d_hi

        do_cast(0)
        do_cast(1)

        # ---- streaming phase (software-pipelined, casts 2 ahead) -------
        for r in range(NT):
            if r + 2 < NT:
                a_f = afp.tile([P, N], f32, tag="af", name=f"af{r+2}")
                strip_q(r).dma_start(out=a_f,
                                     in_=adj[(r + 2) * P:(r + 3) * P, :])
                afs.append(a_f)
                x_t = small.tile([P, F], f32, tag="xt", name=f"xt{r+2}", bufs=6)
                strip_q(r + 1).dma_start(out=x_t, in_=x[(r + 2) * P:(r + 3) * P, :])
                xts.append(x_t)
                do_cast(r + 2)

            a_t = ats_list[r]
            nc.scalar.activation(out=dsq[:, r:r + 1], in_=d_los[r],
                                 func=AF.Sqrt, bias=d_his[r])
            nc.vector.reciprocal(out=dinv[:, r:r + 1], in_=dsq[:, r:r + 1])
            nc.vector.tensor_scalar(out=y0[:, r, :], in0=xts[r],
                                    scalar1=dinv[:, r:r + 1], scalar2=None,
                                    op0=OP.mult)
            nc.vector.tensor_scalar(out=y0d[:, r, :], in0=xts[r],
                                    scalar1=dsq[:, r:r + 1], scalar2=None,
                                    op0=OP.mult)

            # transposes: 2 groups of 8 tiles (one full-bank evac each)
            for g in range(2):
                pt = ps_t.tile([P, 8, P], f32, tag="t")
                for q in range(8):
                    c = 8 * g + q
                    nc.tensor.matmul(pt[:, q, :], lhsT=a_t[:, c * P:(c + 1) * P],
                                     rhs=ident_hf, start=True, stop=True)
                evac(g % 2 == (r % 2), ats[:, 8 * g:8 * g + 8, r * P:(r + 1) * P],
                     pt)

            # pass-1 spread: this strip vs all older chunks
            bi = bnd_of_strip[r]
            for mb in range(bi):
                mo, mw = BUNDLES[mb][0] * P, BUNDLES[mb][1] * P
                nc.tensor.matmul(z1t[:, mo:mo + mw], lhsT=y0[:, r, :],
                                 rhs=ats[:, r, mo:mo + mw],
                                 start=False, stop=(r == NT - 1),
                                 skip_group_check=True)

            # bundle end: y0d transposes + (all strips so far) x new chunk
            blo, bn = BUNDLES[bi]
            if r == blo + bn - 1:
                pty = ps_t.tile([P, 8, P], f32, tag="t")
                for q in range(bn):
                    nc.tensor.matmul(pty[:, q, :], lhsT=y0d[:, blo + q, :],
                                     rhs=ident_hf, start=True, stop=True)
                evac(bi % 2 == 0,
                     y0dT[:, blo * P:(blo + bn) * P].rearrange(
                         "p (b f) -> p b f", b=bn),
                     pty[:, 0:bn, :])

                co, cw = blo * P, bn * P
                for c in range(blo + bn):
                    nc.tensor.matmul(z1t[:, co:co + cw], lhsT=y0[:, c, :],
                                     rhs=ats[:, c, co:co + cw],
                                     start=(c == 0),
                                     stop=(c == NT - 1 and r == NT - 1),
                                     skip_group_check=True)

        # ---- tail ------------------------------------------------------
        nc.vector.scalar_tensor_tensor(out=ndinv2, in0=dinv, scalar=-1.0,
                                       in1=dinv, op0=OP.mult, op1=OP.mult)

        # keep the HAM clock warm across the z1-finish chain
        warm2 = ps_t.tile([P, 8, P], f32, tag="t")
        for _ in range(12):
            nc.tensor.matmul(warm2[:, 0, :], lhsT=ident_hf, rhs=ident_hf,
                             start=True, stop=True)

        # z1 -> fp16 in two big halves
        evac(False, z1bf[:, 0:1024], z1t[:, 0:1024])
        evac(True, z1bf[:, 1024:2048], z1t[:, 1024:2048])
        for m in range(2):
            pt = ps_t.tile([P, 8, P], f32, tag="t")
            for q in range(8):
                nc.tensor.matmul(pt[:, q, :],
                                 lhsT=z1bf[:, (8 * m + q) * P:(8 * m + q + 1) * P],
                                 rhs=ident_hf, start=True, stop=True)
            for q in range(8):
                r = 8 * m + q
                nc.vector.scalar_tensor_tensor(
                    out=y1[:, r, :], in0=pt[:, q, :], scalar=ndinv2[:, r:r + 1],
                    in1=y0[:, r, :], op0=OP.mult, op1=OP.add)

        # force the HAM clock warm right before the dense pass-2 block
        warm3 = ps_t.tile([P, 8, P], f32, tag="t")
        for _ in range(26):
            nc.tensor.matmul(warm3[:, 0, :], lhsT=ident_hf, rhs=ident_hf,
                             start=True, stop=True)

        # pass 2: all 64 matmuls dense; evacs at chunk boundaries
        z2t = ps_acc.tile([P, N], f32, tag="acc")
        for m in range(NB2):
            for c in range(NT):
                nc.tensor.matmul(z2t[:, m * 512:(m + 1) * 512],
                                 lhsT=y1[:, c, :],
                                 rhs=ats[:, c, m * 512:(m + 1) * 512],
                                 start=(c == 0), stop=(c == NT - 1),
                                 skip_group_check=True)
            evac(m % 2 == 1, z2bf[:, m * 512:(m + 1) * 512],
                 z2t[:, m * 512:(m + 1) * 512])

        # Q chunks
        for m in range(NB2):
            qp = ps_t.tile([P, 8, P], f32, tag="t")
            nc.tensor.matmul(qp[:, 0:4, :], lhsT=w_hf[:, 0, :],
                             rhs=y0dT[:, m * 512:(m + 1) * 512],
                             start=True, stop=False)
            nc.tensor.matmul(qp[:, 0:4, :], lhsT=w_hf[:, 1, :],
                             rhs=z1bf[:, m * 512:(m + 1) * 512],
                             start=False, stop=False)
            nc.tensor.matmul(qp[:, 0:4, :], lhsT=w_hf[:, 2, :],
                             rhs=z2bf[:, m * 512:(m + 1) * 512],
                             start=False, stop=True)
            evac(m % 2 == 0,
                 qbf[:, m * 512:(m + 1) * 512].rearrange("p (b f) -> p b f", b=4),
                 qp[:, 0:4, :])

        # q natural + combine + store, in 2 halves of 8 blocks
        for h in range(2):
            ptq = ps_t.tile([P, 8, P], f32, tag="t")
            for q in range(8):
                nc.tensor.matmul(ptq[:, q, :],
                                 lhsT=qbf[:, (8 * h + q) * P:(8 * h + q + 1) * P],
                                 rhs=ident_hf, start=True, stop=True)
            og = small.tile([P, 8, F], f32, tag="og", bufs=2)
            for q in range(8):
                r = 8 * h + q
                tmp = small.tile([P, F], f32, tag="tmp")
                nc.vector.scalar_tensor_tensor(
                    out=tmp, in0=ptq[:, q, :], scalar=dinv[:, r:r + 1],
                    in1=bsum, op0=OP.mult, op1=OP.add)
                nc.scalar.activation(out=og[:, q, :], in_=tmp, func=AF.Relu)
            (nc.sync if h == 0 else nc.scalar).dma_start(
                out=out_t[:, 8 * h:8 * h + 8, :], in_=og)

    nc.compile()
    return nc


def _get_nc():
    if "nc" not in _cache:
        _cache["nc"] = _build_nc()
    return _cache["nc"]


def make_in_maps(x, adj, W, b):
    ident = np.eye(P, dtype=np.float32)
    x = np.ascontiguousarray(np.asarray(x, dtype=np.float32))
    adj = np.ascontiguousarray(np.asarray(adj, dtype=np.float32))
    Wf = np.asarray(W, dtype=np.float32)
    bf = np.asarray(b, dtype=np.float32)
    Wt = np.ascontiguousarray(np.stack(
        [Wf[0] + Wf[1] + Wf[2], -(Wf[1] + 2.0 * Wf[2]), -2.0 * Wf[2]]))
    bsum = np.ascontiguousarray(
        np.broadcast_to(bf.sum(axis=0), (P, F)).astype(np.float32))
    return [
        {"adj": adj[c], "x": x[c], "Wt": Wt, "bsum": bsum, "ident": ident}
        for c in range(NCORES)
    ]


def run_raw(x, adj, W, b, **kwargs):
    from concourse import bass_utils

    nc = _get_nc()
    in_maps = make_in_maps(x, adj, W, b)
    res = bass_utils.run_bass_kernel_spmd(nc, in_maps,
                                          core_ids=list(range(NCORES)), **kwargs)
    out = np.stack([res.results[c]["out"] for c in range(NCORES)], axis=0)
    return out.astype(np.float32), res


def kernel(x, adj, W, b):
    out, _ = run_raw(x, adj, W, b)
    return out
